# revision 39
# baseline (speedup 1.0000x reference)
"""GATv2 (2-layer, 8-head) Trainium2 kernel, 8-core node-sharded.

Pipeline per layer:
  T-NEFF (per-core, sharded): node transforms xl = x@Wl+bl, xr = x@Wr+br
    via fp32r matmuls; emits bf16 gather tables (xl) and bf16 xr shards.
  host: assembles the full xl gather table from the 8 shards (data movement
    only), then
  E-NEFF (per-core, sharded by dst): per-edge score + segment-softmax +
    aggregate, with edges laid out stratum-major: edge slot (q, d) holds the
    q-th in-edge of dst-slot d, so partition index == dst slot.  The
    xr broadcast is a plain broadcast AP, segment aggregation is a PSUM
    accumulation of identity matmuls, and segment max/sum are free-dim
    reduces.  xl[src] rows are fetched with gpsimd dma_gather (int16
    indices, so the node table is split at 32768 and each block gathers
    from both halves into disjoint strata).

Between the two layers the host only concatenates/transposes shards.
"""

import os
from contextlib import ExitStack

import ml_dtypes
import numpy as np

N, E0, DIN, H, DH, DOUT = 50000, 1600000, 128, 8, 16, 7
F1 = H * DH            # 128
F2P = 64               # layer-2 per-node feature block, 8 heads x 8 (7 real)
NCORES = 8
P = 128
NBLK = 392             # 392*128 = 50176 >= N, 392 % 8 == 0
NB = NBLK // NCORES    # 49 blocks per core
NOWN = NB * P          # 6272 nodes per core (incl. pad slots)
NPAD = NBLK * P        # 50176
SPLIT = 32768
TABB_ROWS = NPAD - SPLIT  # 17408
NEG = -60.0  # mask for padded strata; scores are O(10) so exp(-60+s) == 0
EPS = 1e-16

_f32 = np.float32
_bf16 = ml_dtypes.bfloat16


# ---------------------------------------------------------------------------
# host-side graph preprocessing (pure index/layout manipulation)
# ---------------------------------------------------------------------------

def _prep_graph(edge_index):
    src = np.concatenate([edge_index[0], np.arange(N, dtype=np.int64)])
    dst = np.concatenate([edge_index[1], np.arange(N, dtype=np.int64)])
    src = src.astype(np.int64)
    dst = dst.astype(np.int64)

    low = src < SPLIT
    l_cnt = np.bincount(dst[low], minlength=N).astype(np.int64)
    h_cnt = np.bincount(dst[~low], minlength=N).astype(np.int64)

    # group nodes into blocks of 128 with near-equal (low-deg, high-deg):
    # primary sort by low-half in-degree, then re-sort h within coarse
    # l-bands so both per-window maxima stay tight (pads sumG 1994->1854)
    order = np.lexsort((h_cnt, l_cnt))
    BAND = 8192
    parts = []
    for s in range(0, N, BAND):
        seg = order[s:s + BAND]
        parts.append(seg[np.argsort(h_cnt[seg], kind="stable")])
    order = np.concatenate(parts)
    nodes_sorted = np.concatenate([order, np.full(NPAD - N, -1, np.int64)])
    blocks = nodes_sorted.reshape(NBLK, P)          # [392, 128]

    l_blk = np.where(blocks >= 0, l_cnt[np.maximum(blocks, 0)], 0).max(axis=1)
    h_blk = np.where(blocks >= 0, h_cnt[np.maximum(blocks, 0)], 0).max(axis=1)
    # block-slot j on every core runs global blocks j*8+k; shared strata counts
    GA = l_blk.reshape(NB, NCORES).max(axis=1).astype(int)   # [49]
    GB = h_blk.reshape(NB, NCORES).max(axis=1).astype(int)
    GA = GA.astype(int)
    GB = GB.astype(int)

    # per-node padded src lists, split by src half
    key = dst * 2 + (~low).astype(np.int64)
    oe = np.argsort(key, kind="stable")
    ss, sk = src[oe], key[oe]
    starts = np.searchsorted(sk, np.arange(2 * N))
    pos = np.arange(len(ss)) - starts[sk]
    Amax = max(int(l_cnt.max()), int(GA.max()))
    Bmax = max(int(h_cnt.max()), int(GB.max()))
    A_pad = np.zeros((N, Amax), np.int32)
    B_pad = np.zeros((N, Bmax), np.int32)
    am = (sk % 2) == 0
    A_pad[sk[am] // 2, pos[am]] = ss[am]
    B_pad[sk[~am] // 2, pos[~am]] = ss[~am] - SPLIT

    sumGA, sumGB = int(GA.sum()), int(GB.sum())
    sumG = sumGA + sumGB

    members = [None] * NCORES
    idxA = [None] * NCORES
    idxB = [None] * NCORES
    mneg = [None] * NCORES

    for k in range(NCORES):
        mem = blocks[np.arange(NB) * NCORES + k]       # [49, 128]
        members[k] = mem
        ia = np.zeros((P, 8 * sumGA), np.int16)
        ib = np.zeros((P, 8 * sumGB), np.int16)
        mg = np.full((P, sumG), NEG, _f32)
        oa = ob = om = 0
        for j in range(NB):
            ga, gb = GA[j], GB[j]
            m = mem[j]
            msafe = np.maximum(m, 0)
            larr = np.where(m >= 0, l_cnt[msafe], 0)
            harr = np.where(m >= 0, h_cnt[msafe], 0)
            if ga:
                plane = A_pad[msafe, :ga]              # [128, ga] (d, q)
                flat = plane.T.reshape(-1)             # slot-major (q, d)
                ia[:, 8 * oa:8 * (oa + ga)] = np.tile(
                    flat.reshape(-1, 16).T, (8, 1)).astype(np.int16)
                mg[:, om:om + ga] = np.where(
                    np.arange(ga)[None, :] < larr[:, None], 0.0, NEG)
            if gb:
                plane = B_pad[msafe, :gb]
                flat = plane.T.reshape(-1)
                ib[:, 8 * ob:8 * (ob + gb)] = np.tile(
                    flat.reshape(-1, 16).T, (8, 1)).astype(np.int16)
                mg[:, om + ga:om + ga + gb] = np.where(
                    np.arange(gb)[None, :] < harr[:, None], 0.0, NEG)
            oa += ga
            ob += gb
            om += ga + gb
        idxA[k], idxB[k], mneg[k] = ia, ib, mg

    return dict(members=members, GA=GA, GB=GB, idxA=idxA, idxB=idxB,
                mneg=mneg, sumGA=sumGA, sumGB=sumGB, sumG=sumG)


# ---------------------------------------------------------------------------
# NEFF builders
# ---------------------------------------------------------------------------

def _mk_bass(num_swdge_queues=1):
    import concourse.bacc as bacc
    return bacc.Bacc("TRN2", target_bir_lowering=False,
                     num_swdge_queues=num_swdge_queues)


def _build_transform(fo, xl_cols, xl_w, xr_w, elu_in):
    """xT [128, NOWN] (bf16) @ Wcat [128, fo] -> xl rows + xr rows (bf16).

    xl tensor is [NOWN, xl_cols]; only cols [0:xl_w] are written (rest
    stays zero).  xr tensor is [NOWN, xr_w].  With elu_in, the input is
    layer-1's raw (pre-activation) output h and the matmul consumes
    elu(h)+1 -- the host subtracts ones@W from the bias to compensate."""
    import concourse.mybir as mybir
    import concourse.tile as tile

    nc = _mk_bass()
    BF16, F32 = mybir.dt.bfloat16, mybir.dt.float32
    AF = mybir.ActivationFunctionType
    op = mybir.AluOpType
    xT = nc.dram_tensor("xT", [P, NOWN], BF16, kind="ExternalInput")
    W = nc.dram_tensor("Wcat", [P, fo], BF16, kind="ExternalInput")
    B = nc.dram_tensor("Bcat", [P, fo], F32, kind="ExternalInput")
    xl = nc.dram_tensor("xl", [NOWN, xl_cols], BF16, kind="ExternalOutput")
    xr = nc.dram_tensor("xr", [NOWN, xr_w], BF16, kind="ExternalOutput")

    with tile.TileContext(nc) as tc, ExitStack() as ctx:
        const = ctx.enter_context(tc.tile_pool(name="const", bufs=1))
        work = ctx.enter_context(tc.tile_pool(name="work", bufs=3))
        psum = ctx.enter_context(tc.tile_pool(name="psum", bufs=2, space="PSUM"))

        w_s = const.tile([P, fo], BF16)
        nc.sync.dma_start(w_s[:], W[:, :])
        b_s = const.tile([P, fo], F32)
        nc.sync.dma_start(b_s[:], B[:, :])

        for j in range(NB):
            lhs = work.tile([P, P], BF16, tag="lhs")
            nc.sync.dma_start(lhs[:], xT[:, j * P:(j + 1) * P])
            if elu_in:
                # elu(h)+1 = max(h,0) + exp(min(h,0))
                mm = work.tile([P, P], BF16, tag="mm")
                nc.vector.tensor_scalar_min(mm[:], lhs[:], 0.0)
                em = work.tile([P, P], BF16, tag="em")
                nc.scalar.activation(em[:], mm[:], AF.Exp)
                lhs2 = work.tile([P, P], BF16, tag="lhs2")
                nc.vector.scalar_tensor_tensor(lhs2[:], lhs[:], 0.0, em[:],
                                               op.max, op.add)
                lhs = lhs2
            ps = psum.tile([P, fo], F32, tag="ps")
            nc.tensor.matmul(ps[:], lhs[:], w_s[:], start=True, stop=True)
            ol = work.tile([P, xl_w], BF16, tag="ol")
            nc.vector.tensor_tensor(ol[:], ps[:, 0:xl_w], b_s[:, 0:xl_w],
                                    op.add)
            orr = work.tile([P, xr_w], BF16, tag="orr")
            nc.vector.tensor_tensor(orr[:], ps[:, xl_w:fo], b_s[:, xl_w:fo],
                                    op.add)
            nc.sync.dma_start(xl[j * P:(j + 1) * P, 0:xl_w], ol[:])
            nc.sync.dma_start(xr[j * P:(j + 1) * P, :], orr[:])
    nc.compile()
    return nc


def _build_edge(layer, GA, GB, sumGA, sumGB, sumG,
                no_tail=False, repeat=1, no_gather=False, no_score=False):
    """Edge phase for one layer (see module docstring).  no_tail/repeat/
    no_gather/no_score are timing-diagnostic variants (wrong results)."""
    import concourse.bass as bass
    import concourse.mybir as mybir
    import concourse.tile as tile
    from concourse import library_config

    FU = F1 if layer == 1 else F2P      # used feature cols (128 / 64)
    C = DH if layer == 1 else 8         # per-head cols in slab (16 / 8)
    FM = FU + H                         # matmul rhs cols (agg | denom)
    FOUT = F1 if layer == 1 else H * DOUT

    # 4 SWDGE queues: dma_gather descriptor generation runs on the Q7 core
    # pair (2q, 2q+1) selected by queue_num -- round-robinning the gathers
    # across queues 0-3 parallelizes descgen over all 8 Q7 cores instead of
    # serializing on cores 0/1
    nc = _mk_bass(num_swdge_queues=4)
    dt = mybir.dt
    op = mybir.AluOpType
    AF = mybir.ActivationFunctionType

    tabA = nc.dram_tensor("tabA", [SPLIT, P], dt.bfloat16, kind="ExternalInput")
    tabB = nc.dram_tensor("tabB", [TABB_ROWS, P], dt.bfloat16, kind="ExternalInput")
    xr_d = nc.dram_tensor("xr", [NOWN, FU], dt.bfloat16, kind="ExternalInput")
    idxA = nc.dram_tensor("idxA", [P, 8 * sumGA], dt.int16, kind="ExternalInput")
    idxB = nc.dram_tensor("idxB", [P, 8 * sumGB], dt.int16, kind="ExternalInput")
    mneg = nc.dram_tensor("mneg", [P, sumG], dt.float32, kind="ExternalInput")
    attT = nc.dram_tensor("attT", [P, FU], dt.bfloat16, kind="ExternalInput")
    biasT = nc.dram_tensor("biasT", [P, FU], dt.float32, kind="ExternalInput")
    idT = nc.dram_tensor("idT", [P, P], dt.bfloat16, kind="ExternalInput")
    out_dt = dt.bfloat16 if layer == 1 else dt.float32
    out_d = nc.dram_tensor("out", [NOWN, FOUT], out_dt, kind="ExternalOutput")

    with tile.TileContext(nc) as tc, ExitStack() as ctx:
        const = ctx.enter_context(tc.tile_pool(name="const", bufs=1))
        io = ctx.enter_context(tc.tile_pool(name="io", bufs=4))
        slabp = ctx.enter_context(tc.tile_pool(name="slabp", bufs=3))
        slabs = ctx.enter_context(tc.tile_pool(name="slabs", bufs=2))
        psum = ctx.enter_context(tc.tile_pool(name="psum", bufs=2, space="PSUM"))
        small = ctx.enter_context(tc.tile_pool(name="small", bufs=2))

        nc.gpsimd.load_library(library_config.mlp)

        regcache = {}

        def nreg(v):
            if v not in regcache:
                regcache[v] = nc.gpsimd.to_reg(v)
            return regcache[v]

        att_s = const.tile([P, FU], dt.bfloat16)
        nc.sync.dma_start(att_s[:], attT[:, :])
        bias_s = const.tile([P, FU], dt.float32)
        nc.sync.dma_start(bias_s[:], biasT[:, :])
        id_s = const.tile([P, P], dt.bfloat16)
        nc.sync.dma_start(id_s[:], idT[:, :])

        if layer == 2:
            persist = ctx.enter_context(tc.tile_pool(name="persist", bufs=1))
            s_all = persist.tile([P, NB], dt.float32)
            y_all = persist.tile([P, NB * repeat, FU], dt.float32,
                                 name="y_all")

        oa = obi = om = orow = 0
        for j0 in range(NB * repeat):
            j = j0 % NB
            if j == 0:
                oa = obi = om = orow = 0
            ga, gb = int(GA[j]), int(GB[j])
            g = ga + gb
            assert g > 0

            xr_b = io.tile([P, FU], dt.bfloat16, tag="xr")
            nc.sync.dma_start(xr_b[:], xr_d[j * P:(j + 1) * P, :])
            mg = io.tile([P, g], dt.float32, tag="mg")
            nc.sync.dma_start(mg[:], mneg[:, om:om + g])

            slab = slabp.tile([P, g, P], dt.bfloat16, tag="slab")
            if ga and not no_gather:
                ia = io.tile([P, 8 * ga], dt.int16, tag="ia")
                nc.sync.dma_start(ia[:], idxA[:, 8 * oa:8 * (oa + ga)])
                nc.gpsimd.dma_gather(slab[:, 0:ga, :], tabA[:, :], ia[:],
                                     P * ga, nreg(P * ga), P,
                                     single_packet=False,
                                     queue_num=(2 * j) % 4)
            if gb and not no_gather:
                ib = io.tile([P, 8 * gb], dt.int16, tag="ib")
                nc.sync.dma_start(ib[:], idxB[:, 8 * obi:8 * (obi + gb)])
                nc.gpsimd.dma_gather(slab[:, ga:g, :], tabB[:, :], ib[:],
                                     P * gb, nreg(P * gb), P,
                                     single_packet=False,
                                     queue_num=(2 * j + 1) % 4)
            if no_gather:
                nc.vector.memset(slab[:, 0:1, :], 0.5)

            sl_u = slab[:, :, 0:FU]
            Ms = slabs.tile([P, g, FM], dt.bfloat16, tag="Ms")
            exv = Ms[:, :, FU:FM]
            if no_score:
                nc.vector.memset(exv, 1.0)
            else:
                tt = slabs.tile([P, g, FU], dt.bfloat16, tag="tt")
                nc.vector.tensor_tensor(
                    tt[:], sl_u,
                    xr_b[:].unsqueeze(1).to_broadcast([P, g, FU]),
                    op.add)
                # leaky-relu on the Scalar engine, in place (frees a DVE
                # pass and a tile). ACT's Lrelu hardwires slope 0.01
                # (ignores alpha); Prelu honors it
                nc.scalar.activation(tt[:], tt[:], AF.Prelu, alpha=0.2)
                # vv and the halving tree run in fp16: 16-bit keeps the DVE
                # 2x mode and the 10-bit mantissa keeps partial-sum rounding
                # out of the scores (values here are O(10), no range risk)
                vv = slabs.tile([P, g, FU], dt.float16, tag="vv")
                nc.vector.tensor_tensor(
                    vv[:], tt[:],
                    att_s[:].unsqueeze(1).to_broadcast([P, g, FU]),
                    op.mult)

                # per-head sum over C via a tensor_tensor halving tree (2x)
                # instead of a 1x tensor_reduce; levels below the first halve
                # in place within tr
                vh = vv[:].rearrange("p g (h c) -> p g h c", c=C)
                tr = slabs.tile([P, g, H, C // 2], dt.float16, tag="tr")
                nc.vector.tensor_tensor(
                    tr[:], vh[:, :, :, 0:C // 2], vh[:, :, :, C // 2:C],
                    op.add)
                lv = C // 2
                cur = tr[:]
                while lv > 2:
                    nc.vector.tensor_tensor(
                        cur[:, :, :, 0:lv // 2], cur[:, :, :, 0:lv // 2],
                        cur[:, :, :, lv // 2:lv], op.add)
                    cur = cur
                    lv //= 2
                cur = cur[:, :, :, 0:2]
                sc2 = small.tile([P, g, H], dt.float32, tag="sc2")
                scf = small.tile([P, g, H], dt.float32, tag="scf")
                nc.vector.tensor_tensor(
                    scf[:].unsqueeze(3), cur[:, :, :, 0:1], cur[:, :, :, 1:2],
                    op.add)
                nc.vector.tensor_tensor(
                    sc2[:], scf[:], mg[:].unsqueeze(2).to_broadcast([P, g, H]),
                    op.add)

                # scores are O(10) bounded -> exp without max subtraction
                nc.scalar.activation(exv, sc2[:], AF.Exp)
            nc.vector.tensor_tensor(
                Ms[:, :, 0:FU].rearrange("p g (h c) -> p g h c", c=C),
                sl_u.rearrange("p g (h c) -> p g h c", c=C),
                exv.unsqueeze(3).to_broadcast([P, g, H, C]),
                op.mult)

            # aggregate strata in groups of QG per matmul (wider rhs keeps
            # the PE streaming instead of paying per-instruction overhead)
            QG = 3 if layer == 1 else 7
            ngrp = min(QG, g)
            ps = psum.tile([P, ngrp, FM], dt.float32, tag="ps")
            nmm = (g + ngrp - 1) // ngrp
            for i in range(nmm):
                q0 = i * ngrp
                w = min(ngrp, g - q0)
                nc.tensor.matmul(ps[:, 0:w, :], id_s[:],
                                 Ms[:, q0:q0 + w, :],
                                 start=(i == 0), stop=(i == nmm - 1))
            # combine the ngrp partial groups (note: strata counts are even
            # multiples... tail group only wrote w<=ngrp slots on its last
            # matmul, but those slots were fully accumulated on earlier
            # passes, so every slot is valid)
            acc = small.tile([P, FM], dt.float32, tag="acc")
            nc.vector.tensor_copy(acc[:], ps[:, 0, :])
            for k in range(1, ngrp):
                nc.vector.tensor_tensor(acc[:], acc[:], ps[:, k, :], op.add)

            dn = small.tile([P, H], dt.float32, tag="dn")
            nc.vector.tensor_scalar_add(dn[:], acc[:, FU:FM], EPS)
            rd = small.tile([P, H], dt.float32, tag="rd")
            nc.vector.reciprocal(rd[:], dn[:])
            ov = small.tile([P, FU], dt.float32, tag="ov")
            nc.vector.tensor_tensor(
                ov[:].rearrange("p (h c) -> p h c", c=C),
                acc[:, 0:FU].rearrange("p (h c) -> p h c", c=C),
                rd[:].unsqueeze(2).to_broadcast([P, H, C]),
                op.mult)
            if layer == 2 and not no_tail:
                # bias-add writes straight into the persistent y buffer; all
                # log-softmax work happens batched after the loop
                nc.vector.tensor_tensor(y_all[:, j0, :], ov[:], bias_s[:],
                                        op.add)
            else:
                ob = small.tile([P, FU], out_dt, tag="ob")
                nc.vector.tensor_tensor(ob[:], ov[:], bias_s[:], op.add)
                nc.sync.dma_start(out_d[orow:orow + P, :], ob[:, 0:FOUT])

            oa += ga
            obi += gb
            om += g
            orow += P

        if layer == 2 and not no_tail:
            # batched log-softmax over all blocks: y_all holds [P, NB, 64]
            ya4 = y_all[:, 0:NB, :].rearrange("p n (h c) -> p n h c", c=8)
            yr = ya4[:, :, :, 0:DOUT]                  # [P, NB, H, 7]
            mx_all = persist.tile([P, NB], dt.float32)
            nc.vector.tensor_reduce(mx_all[:], yr, mybir.AxisListType.XY,
                                    op.max)
            ysub = persist.tile([P, NB, FU], dt.float32, name="ysub")
            nc.vector.tensor_tensor(
                ysub[:], y_all[:, 0:NB, :],
                mx_all[:].unsqueeze(2).to_broadcast([P, NB, FU]),
                op.subtract)
            et_all = persist.tile([P, NB, FU], dt.float32, name="et_all")
            nc.scalar.activation(et_all[:], ysub[:], AF.Exp)
            er4 = et_all[:].rearrange("p n (h c) -> p n h c", c=8)
            nc.vector.tensor_reduce(s_all[:], er4[:, :, :, 0:DOUT],
                                    mybir.AxisListType.XY, op.add)
            # ln(S) via exponent/mantissa split (no Ln in any HW act table):
            # ln(S) = (e - 127)*ln2 + poly(m), m in [1, 2)
            C5, C4, C3, C2, C1, C0 = (0.030102625011658456,
                                      -0.2806325404494927,
                                      1.1048082361987304,
                                      -2.4208125632180866,
                                      3.4982279012091095,
                                      -1.9316715417207186)
            bits = s_all[:].bitcast(dt.int32)
            ei = persist.tile([P, NB], dt.int32)
            nc.vector.tensor_scalar(ei[:], bits, 23, None,
                                    op.arith_shift_right)
            ef = persist.tile([P, NB], dt.float32)
            nc.vector.tensor_copy(ef[:], ei[:])
            mi = persist.tile([P, NB], dt.int32)
            nc.vector.tensor_scalar(mi[:], bits, 0x007FFFFF, 0x3F800000,
                                    op.bitwise_and, op.bitwise_or)
            mf = mi[:].bitcast(dt.float32)
            pp = persist.tile([P, NB], dt.float32)
            nc.vector.tensor_scalar(pp[:], mf, C5, C4, op.mult, op.add)
            qq = persist.tile([P, NB], dt.float32)
            for ck in (C3, C2, C1, C0):
                nc.vector.tensor_tensor(qq[:], pp[:], mf, op.mult)
                nc.vector.tensor_scalar_add(pp[:], qq[:], ck)
            # ct = mx + (e-127)*ln2 + poly(m)
            lnm = pp
            ct_all = persist.tile([P, NB], dt.float32)
            nc.vector.scalar_tensor_tensor(
                ct_all[:], ef[:], 0.6931471805599453, lnm[:],
                op.mult, op.add)
            lnS = persist.tile([P, NB], dt.float32)
            nc.vector.tensor_scalar_add(lnS[:], ct_all[:],
                                        -127.0 * 0.6931471805599453)
            # ysub already has -mx; subtract only ln(S)
            of_all = persist.tile([P, NB, FOUT], dt.float32, name="of_all")
            nc.vector.tensor_tensor(
                of_all[:].rearrange("p n (h c) -> p n h c", c=DOUT),
                ysub[:].rearrange("p n (h c) -> p n h c", c=8)[:, :, :, 0:DOUT],
                lnS[:].unsqueeze(2).unsqueeze(3).to_broadcast(
                    [P, NB, H, DOUT]),
                op.subtract)
            for j in range(NB):
                nc.sync.dma_start(out_d[j * P:(j + 1) * P, :],
                                  of_all[:, j, :])
    nc.compile()
    return nc


# ---------------------------------------------------------------------------
# runner
# ---------------------------------------------------------------------------

_state = {}


def _run(nc, in_maps, trace=False):
    from concourse.bass_utils import run_bass_kernel_spmd
    return run_bass_kernel_spmd(nc, in_maps, core_ids=list(range(NCORES)),
                                trace=trace)


def _bench_run(nc, in_maps):
    import bench
    results, wall, walls = bench.bench_neff(nc, in_maps)
    return results, wall, walls


def _bcast_rows(v, rows=P):
    """[n] -> [rows, n] replicated, contiguous."""
    return np.ascontiguousarray(np.broadcast_to(np.asarray(v)[None, :],
                                                (rows, len(v))))


def kernel(x, edge_index, Wl1, bl1, Wr1, br1, att1, bias1,
           Wl2, bl2, Wr2, br2, att2, bias2, _bench=False, _times=None,
           _walls=None):
    x = np.asarray(x, _f32)
    edge_index = np.asarray(edge_index)

    g = _prep_graph(edge_index)
    members, GA, GB = g["members"], g["GA"], g["GB"]

    ckey = (tuple(GA), tuple(GB))
    if _state.get("ckey") != ckey:
        _state["ckey"] = ckey
        _state["nc_t1"] = _build_transform(2 * F1, F1, F1, F1, elu_in=False)
        _state["nc_t2"] = _build_transform(2 * F2P, P, F2P, F2P, elu_in=True)
        _state["nc_e1"] = _build_edge(1, GA, GB, g["sumGA"], g["sumGB"], g["sumG"])
        _state["nc_e2"] = _build_edge(2, GA, GB, g["sumGA"], g["sumGB"], g["sumG"])

    id128 = np.eye(P, dtype=_bf16)

    def gather_nodes(arr, mem):
        flat = mem.reshape(-1)
        out = arr[np.maximum(flat, 0)]
        out[flat < 0] = 0
        return out

    def trace_run(key, nc, in_maps):
        if _bench:
            results, wall, wl = _bench_run(nc, in_maps)
            if _times is not None:
                _times[key] = wall
            if _walls is not None:
                _walls[key] = wl
            return results
        r = _run(nc, in_maps, trace=False)
        return r.results

    # ---- T1 ----
    W1 = np.concatenate([Wl1, Wr1], axis=1).astype(_bf16)      # [128, 256]
    B1 = np.concatenate([bl1, br1]).astype(_f32)               # [256]
    B1t = _bcast_rows(B1)
    t1_maps = []
    for k in range(NCORES):
        xg = gather_nodes(x, members[k])                       # [6272, 128]
        t1_maps.append({"xT": np.ascontiguousarray(xg.T).astype(_bf16),
                        "Wcat": W1, "Bcat": B1t})
    r1 = trace_run("t1", _state["nc_t1"], t1_maps)

    # assemble layer-1 gather table
    tab1 = np.zeros((NPAD, P), _bf16)
    for k in range(NCORES):
        flat = members[k].reshape(-1)
        ok = flat >= 0
        tab1[flat[ok]] = r1[k]["xl"][ok]
    tab1A = np.ascontiguousarray(tab1[:SPLIT])
    tab1B = np.ascontiguousarray(tab1[SPLIT:])

    # ---- E1 ----
    att1_t = _bcast_rows(att1.reshape(-1)).astype(_bf16)       # [128, 128]
    bias1_t = _bcast_rows(bias1).astype(_f32)
    e1_maps = []
    for k in range(NCORES):
        e1_maps.append({"tabA": tab1A, "tabB": tab1B,
                        "xr": r1[k]["xr"],
                        "idxA": g["idxA"][k], "idxB": g["idxB"][k],
                        "mneg": g["mneg"][k],
                        "attT": att1_t, "biasT": bias1_t, "idT": id128})
    re1 = trace_run("e1", _state["nc_e1"], e1_maps)

    # ---- T2 ----
    Wl2p = np.zeros((P, F2P), _f32)
    Wl2p.reshape(P, H, 8)[:, :, :DOUT] = np.asarray(Wl2, _f32).reshape(P, H, DOUT)
    Wr2p = np.zeros((P, F2P), _f32)
    Wr2p.reshape(P, H, 8)[:, :, :DOUT] = np.asarray(Wr2, _f32).reshape(P, H, DOUT)
    W2 = np.concatenate([Wl2p, Wr2p], axis=1).astype(_bf16)  # [128,128]
    bl2p = np.zeros(F2P, _f32)
    bl2p.reshape(H, 8)[:, :DOUT] = np.asarray(bl2, _f32).reshape(H, DOUT)
    br2p = np.zeros(F2P, _f32)
    br2p.reshape(H, 8)[:, :DOUT] = np.asarray(br2, _f32).reshape(H, DOUT)
    # t2 consumes elu(h)+1, so shift the bias by -ones@W (col sums of the
    # bf16-rounded W as actually used on-device)
    B2 = np.concatenate([bl2p, br2p]) - W2.astype(_f32).sum(axis=0)
    B2t = _bcast_rows(B2)
    t2_maps = []
    for k in range(NCORES):
        t2_maps.append({"xT": np.ascontiguousarray(re1[k]["out"].T),
                        "Wcat": W2, "Bcat": B2t})
    r2 = trace_run("t2", _state["nc_t2"], t2_maps)

    tab2 = np.zeros((NPAD, P), _bf16)
    for k in range(NCORES):
        flat = members[k].reshape(-1)
        ok = flat >= 0
        tab2[flat[ok]] = r2[k]["xl"][ok]
    tab2A = np.ascontiguousarray(tab2[:SPLIT])
    tab2B = np.ascontiguousarray(tab2[SPLIT:])

    # ---- E2 ----
    att2p = np.zeros((H, 8), _f32)
    att2p[:, :DOUT] = np.asarray(att2, _f32)
    att2_t = _bcast_rows(att2p.reshape(-1)).astype(_bf16)      # [128, 64]
    bias2p = np.zeros(F2P, _f32)
    bias2p.reshape(H, 8)[:, :DOUT] = np.asarray(bias2, _f32).reshape(H, DOUT)
    bias2_t = _bcast_rows(bias2p)
    e2_maps = []
    for k in range(NCORES):
        e2_maps.append({"tabA": tab2A, "tabB": tab2B,
                        "xr": r2[k]["xr"],
                        "idxA": g["idxA"][k], "idxB": g["idxB"][k],
                        "mneg": g["mneg"][k],
                        "attT": att2_t, "biasT": bias2_t, "idT": id128})
    re2 = trace_run("e2", _state["nc_e2"], e2_maps)

    out = np.zeros((N, H * DOUT), _f32)
    for k in range(NCORES):
        flat = members[k].reshape(-1)
        ok = flat >= 0
        out[flat[ok]] = re2[k]["out"][ok]
    return out



# revision 47
# speedup vs baseline: 1.2363x; 1.2363x over previous
"""GATv2 (2-layer, 8-head) Trainium2 kernel, 8-core node-sharded.

Pipeline per layer:
  T-NEFF (per-core, sharded): node transforms xl = x@Wl+bl, xr = x@Wr+br
    via fp32r matmuls; emits bf16 gather tables (xl) and bf16 xr shards.
  host: assembles the full xl gather table from the 8 shards (data movement
    only), then
  E-NEFF (per-core, sharded by dst): per-edge score + segment-softmax +
    aggregate, with edges laid out stratum-major: edge slot (q, d) holds the
    q-th in-edge of dst-slot d, so partition index == dst slot.  The
    xr broadcast is a plain broadcast AP, segment aggregation is a PSUM
    accumulation of identity matmuls, and segment max/sum are free-dim
    reduces.  xl[src] rows are fetched with gpsimd dma_gather (int16
    indices, so the node table is split at 32768 and each block gathers
    from both halves into disjoint strata).

Between the two layers the host only concatenates/transposes shards.
"""

import os
from contextlib import ExitStack

import ml_dtypes
import numpy as np

N, E0, DIN, H, DH, DOUT = 50000, 1600000, 128, 8, 16, 7
F1 = H * DH            # 128
F2P = 64               # layer-2 per-node feature block, 8 heads x 8 (7 real)
NCORES = 8
P = 128
NBLK = 392             # 392*128 = 50176 >= N, 392 % 8 == 0
NB = NBLK // NCORES    # 49 blocks per core
NOWN = NB * P          # 6272 nodes per core (incl. pad slots)
NPAD = NBLK * P        # 50176
SPLIT = 32768
TABB_ROWS = NPAD - SPLIT  # 17408
NEG = -60.0  # mask for padded strata; scores are O(10) so exp(-60+s) == 0
EPS = 1e-16

_f32 = np.float32
_bf16 = ml_dtypes.bfloat16


# ---------------------------------------------------------------------------
# host-side graph preprocessing (pure index/layout manipulation)
# ---------------------------------------------------------------------------

def _prep_graph(edge_index):
    src = np.concatenate([edge_index[0], np.arange(N, dtype=np.int64)])
    dst = np.concatenate([edge_index[1], np.arange(N, dtype=np.int64)])
    src = src.astype(np.int64)
    dst = dst.astype(np.int64)

    low = src < SPLIT
    l_cnt = np.bincount(dst[low], minlength=N).astype(np.int64)
    h_cnt = np.bincount(dst[~low], minlength=N).astype(np.int64)

    # group nodes into blocks of 128 with near-equal (low-deg, high-deg):
    # primary sort by low-half in-degree, then re-sort h within coarse
    # l-bands so both per-window maxima stay tight (pads sumG 1994->1854)
    order = np.lexsort((h_cnt, l_cnt))
    BAND = 8192
    parts = []
    for s in range(0, N, BAND):
        seg = order[s:s + BAND]
        parts.append(seg[np.argsort(h_cnt[seg], kind="stable")])
    order = np.concatenate(parts)
    nodes_sorted = np.concatenate([order, np.full(NPAD - N, -1, np.int64)])
    blocks = nodes_sorted.reshape(NBLK, P)          # [392, 128]

    l_blk = np.where(blocks >= 0, l_cnt[np.maximum(blocks, 0)], 0).max(axis=1)
    h_blk = np.where(blocks >= 0, h_cnt[np.maximum(blocks, 0)], 0).max(axis=1)
    # block-slot j on every core runs global blocks j*8+k; shared strata counts
    GA = l_blk.reshape(NB, NCORES).max(axis=1).astype(int)   # [49]
    GB = h_blk.reshape(NB, NCORES).max(axis=1).astype(int)
    GA = GA.astype(int)
    GB = GB.astype(int)

    # per-node padded src lists, split by src half
    key = dst * 2 + (~low).astype(np.int64)
    oe = np.argsort(key, kind="stable")
    ss, sk = src[oe], key[oe]
    starts = np.searchsorted(sk, np.arange(2 * N))
    pos = np.arange(len(ss)) - starts[sk]
    Amax = max(int(l_cnt.max()), int(GA.max()))
    Bmax = max(int(h_cnt.max()), int(GB.max()))
    A_pad = np.zeros((N, Amax), np.int32)
    B_pad = np.zeros((N, Bmax), np.int32)
    am = (sk % 2) == 0
    A_pad[sk[am] // 2, pos[am]] = ss[am]
    B_pad[sk[~am] // 2, pos[~am]] = ss[~am] - SPLIT

    sumGA, sumGB = int(GA.sum()), int(GB.sum())
    sumG = sumGA + sumGB

    members = [None] * NCORES
    idxA = [None] * NCORES
    idxB = [None] * NCORES
    mneg = [None] * NCORES

    for k in range(NCORES):
        mem = blocks[np.arange(NB) * NCORES + k]       # [49, 128]
        members[k] = mem
        ia = np.zeros((P, 8 * sumGA), np.int16)
        ib = np.zeros((P, 8 * sumGB), np.int16)
        mg = np.full((P, sumG), NEG, _f32)
        oa = ob = om = 0
        for j in range(NB):
            ga, gb = GA[j], GB[j]
            m = mem[j]
            msafe = np.maximum(m, 0)
            larr = np.where(m >= 0, l_cnt[msafe], 0)
            harr = np.where(m >= 0, h_cnt[msafe], 0)
            if ga:
                plane = A_pad[msafe, :ga]              # [128, ga] (d, q)
                flat = plane.T.reshape(-1)             # slot-major (q, d)
                ia[:, 8 * oa:8 * (oa + ga)] = np.tile(
                    flat.reshape(-1, 16).T, (8, 1)).astype(np.int16)
                mg[:, om:om + ga] = np.where(
                    np.arange(ga)[None, :] < larr[:, None], 0.0, NEG)
            if gb:
                plane = B_pad[msafe, :gb]
                flat = plane.T.reshape(-1)
                ib[:, 8 * ob:8 * (ob + gb)] = np.tile(
                    flat.reshape(-1, 16).T, (8, 1)).astype(np.int16)
                mg[:, om + ga:om + ga + gb] = np.where(
                    np.arange(gb)[None, :] < harr[:, None], 0.0, NEG)
            oa += ga
            ob += gb
            om += ga + gb
        idxA[k], idxB[k], mneg[k] = ia, ib, mg

    return dict(members=members, GA=GA, GB=GB, idxA=idxA, idxB=idxB,
                mneg=mneg, sumGA=sumGA, sumGB=sumGB, sumG=sumG)


# ---------------------------------------------------------------------------
# NEFF builders
# ---------------------------------------------------------------------------

def _mk_bass(num_swdge_queues=1):
    import concourse.bacc as bacc
    return bacc.Bacc("TRN2", target_bir_lowering=False,
                     num_swdge_queues=num_swdge_queues)


def _build_transform(fo, xl_cols, xl_w, xr_w, elu_in, repeat=1):
    """xT [128, NOWN] (bf16) @ Wcat [128, fo] -> xl rows + xr rows (bf16).

    xl tensor is [NOWN, xl_cols]; only cols [0:xl_w] are written (rest
    stays zero).  xr tensor is [NOWN, xr_w].  With elu_in, the input is
    layer-1's raw (pre-activation) output h and the matmul consumes
    elu(h)+1 -- the host subtracts ones@W from the bias to compensate."""
    import concourse.mybir as mybir
    import concourse.tile as tile

    nc = _mk_bass()
    BF16, F32 = mybir.dt.bfloat16, mybir.dt.float32
    AF = mybir.ActivationFunctionType
    op = mybir.AluOpType
    xT = nc.dram_tensor("xT", [P, NOWN], BF16, kind="ExternalInput")
    W = nc.dram_tensor("Wcat", [P, fo], BF16, kind="ExternalInput")
    B = nc.dram_tensor("Bcat", [P, fo], F32, kind="ExternalInput")
    xl = nc.dram_tensor("xl", [NOWN, xl_cols], BF16, kind="ExternalOutput")
    xr = nc.dram_tensor("xr", [NOWN, xr_w], BF16, kind="ExternalOutput")

    with tile.TileContext(nc) as tc, ExitStack() as ctx:
        const = ctx.enter_context(tc.tile_pool(name="const", bufs=1))
        work = ctx.enter_context(tc.tile_pool(name="work", bufs=3))
        psum = ctx.enter_context(tc.tile_pool(name="psum", bufs=2, space="PSUM"))

        w_s = const.tile([P, fo], BF16)
        nc.sync.dma_start(w_s[:], W[:, :])
        b_s = const.tile([P, fo], F32)
        nc.sync.dma_start(b_s[:], B[:, :])

        for j0 in range(NB * repeat):
            j = j0 % NB
            lhs = work.tile([P, P], BF16, tag="lhs")
            nc.sync.dma_start(lhs[:], xT[:, j * P:(j + 1) * P])
            if elu_in:
                # elu(h)+1 = max(h,0) + exp(min(h,0))
                mm = work.tile([P, P], BF16, tag="mm")
                nc.vector.tensor_scalar_min(mm[:], lhs[:], 0.0)
                em = work.tile([P, P], BF16, tag="em")
                nc.scalar.activation(em[:], mm[:], AF.Exp)
                lhs2 = work.tile([P, P], BF16, tag="lhs2")
                nc.vector.scalar_tensor_tensor(lhs2[:], lhs[:], 0.0, em[:],
                                               op.max, op.add)
                lhs = lhs2
            ps = psum.tile([P, fo], F32, tag="ps")
            nc.tensor.matmul(ps[:], lhs[:], w_s[:], start=True, stop=True)
            ol = work.tile([P, xl_w], BF16, tag="ol")
            nc.vector.tensor_tensor(ol[:], ps[:, 0:xl_w], b_s[:, 0:xl_w],
                                    op.add)
            orr = work.tile([P, xr_w], BF16, tag="orr")
            nc.vector.tensor_tensor(orr[:], ps[:, xl_w:fo], b_s[:, xl_w:fo],
                                    op.add)
            nc.sync.dma_start(xl[j * P:(j + 1) * P, 0:xl_w], ol[:])
            nc.sync.dma_start(xr[j * P:(j + 1) * P, :], orr[:])
    nc.compile()
    return nc


def _build_edge(layer, GA, GB, sumGA, sumGB, sumG,
                no_tail=False, repeat=1, no_gather=False, no_score=False):
    """Edge phase for one layer (see module docstring).  no_tail/repeat/
    no_gather/no_score are timing-diagnostic variants (wrong results)."""
    import concourse.bass as bass
    import concourse.mybir as mybir
    import concourse.tile as tile
    from concourse import library_config

    FU = F1 if layer == 1 else F2P      # used feature cols (128 / 64)
    C = DH if layer == 1 else 8         # per-head cols in slab (16 / 8)
    FM = FU + H                         # matmul rhs cols (agg | denom)
    FOUT = F1 if layer == 1 else H * DOUT

    # 4 SWDGE queues: dma_gather descriptor generation runs on the Q7 core
    # pair (2q, 2q+1) selected by queue_num -- round-robinning the gathers
    # across queues 0-3 parallelizes descgen over all 8 Q7 cores instead of
    # serializing on cores 0/1
    nc = _mk_bass(num_swdge_queues=4)
    dt = mybir.dt
    op = mybir.AluOpType
    AF = mybir.ActivationFunctionType

    tabA = nc.dram_tensor("tabA", [SPLIT, P], dt.bfloat16, kind="ExternalInput")
    tabB = nc.dram_tensor("tabB", [TABB_ROWS, P], dt.bfloat16, kind="ExternalInput")
    xr_d = nc.dram_tensor("xr", [NOWN, FU], dt.bfloat16, kind="ExternalInput")
    idxA = nc.dram_tensor("idxA", [P, 8 * sumGA], dt.int16, kind="ExternalInput")
    idxB = nc.dram_tensor("idxB", [P, 8 * sumGB], dt.int16, kind="ExternalInput")
    mneg = nc.dram_tensor("mneg", [P, sumG], dt.float32, kind="ExternalInput")
    attT = nc.dram_tensor("attT", [P, FU], dt.bfloat16, kind="ExternalInput")
    biasT = nc.dram_tensor("biasT", [P, FU], dt.float32, kind="ExternalInput")
    idT = nc.dram_tensor("idT", [P, P], dt.bfloat16, kind="ExternalInput")
    out_dt = dt.bfloat16 if layer == 1 else dt.float32
    if layer == 1:
        # layer 1 fuses the layer-2 node transform: instead of h it emits
        # xl2 gather-table rows and xr2 directly (t2's NEFF disappears)
        W2c = nc.dram_tensor("W2cat", [P, P], dt.bfloat16,
                             kind="ExternalInput")
        b2T = nc.dram_tensor("bias2T", [P, P], dt.float32,
                             kind="ExternalInput")
        xl2_d = nc.dram_tensor("xl2", [NOWN, P], dt.bfloat16,
                               kind="ExternalOutput")
        xr2_d = nc.dram_tensor("xr2", [NOWN, F2P], dt.bfloat16,
                               kind="ExternalOutput")
        out_d = None
    else:
        out_d = nc.dram_tensor("out", [NOWN, FOUT], out_dt,
                               kind="ExternalOutput")

    with tile.TileContext(nc) as tc, ExitStack() as ctx:
        const = ctx.enter_context(tc.tile_pool(name="const", bufs=1))
        io = ctx.enter_context(tc.tile_pool(name="io", bufs=4))
        slabp = ctx.enter_context(tc.tile_pool(name="slabp", bufs=3))
        slabs = ctx.enter_context(tc.tile_pool(name="slabs", bufs=2))
        psum = ctx.enter_context(tc.tile_pool(name="psum", bufs=2, space="PSUM"))
        small = ctx.enter_context(tc.tile_pool(name="small", bufs=2))

        nc.gpsimd.load_library(library_config.mlp)

        regcache = {}

        def nreg(v):
            if v not in regcache:
                regcache[v] = nc.gpsimd.to_reg(v)
            return regcache[v]

        att_s = const.tile([P, FU], dt.bfloat16)
        nc.sync.dma_start(att_s[:], attT[:, :])
        bias_s = const.tile([P, FU], dt.float32)
        nc.sync.dma_start(bias_s[:], biasT[:, :])
        id_s = const.tile([P, P], dt.bfloat16)
        nc.sync.dma_start(id_s[:], idT[:, :])
        if layer == 1:
            w2_s = const.tile([P, P], dt.bfloat16)
            nc.sync.dma_start(w2_s[:], W2c[:, :])
            b2_s = const.tile([P, P], dt.float32)
            nc.sync.dma_start(b2_s[:], b2T[:, :])

        if layer == 2:
            persist = ctx.enter_context(tc.tile_pool(name="persist", bufs=1))
            s_all = persist.tile([P, NB], dt.float32)
            y_all = persist.tile([P, NB * repeat, FU], dt.float32,
                                 name="y_all")

        oa = obi = om = orow = 0
        for j0 in range(NB * repeat):
            j = j0 % NB
            if j == 0:
                oa = obi = om = orow = 0
            ga, gb = int(GA[j]), int(GB[j])
            g = ga + gb
            assert g > 0

            xr_b = io.tile([P, FU], dt.bfloat16, tag="xr")
            nc.sync.dma_start(xr_b[:], xr_d[j * P:(j + 1) * P, :])
            mg = io.tile([P, g], dt.float32, tag="mg")
            nc.sync.dma_start(mg[:], mneg[:, om:om + g])

            slab = slabp.tile([P, g, P], dt.bfloat16, tag="slab")
            if ga and not no_gather:
                ia = io.tile([P, 8 * ga], dt.int16, tag="ia")
                nc.sync.dma_start(ia[:], idxA[:, 8 * oa:8 * (oa + ga)])
                nc.gpsimd.dma_gather(slab[:, 0:ga, :], tabA[:, :], ia[:],
                                     P * ga, nreg(P * ga), P,
                                     single_packet=False,
                                     queue_num=(2 * j) % 4)
            if gb and not no_gather:
                ib = io.tile([P, 8 * gb], dt.int16, tag="ib")
                nc.sync.dma_start(ib[:], idxB[:, 8 * obi:8 * (obi + gb)])
                nc.gpsimd.dma_gather(slab[:, ga:g, :], tabB[:, :], ib[:],
                                     P * gb, nreg(P * gb), P,
                                     single_packet=False,
                                     queue_num=(2 * j + 1) % 4)
            if no_gather:
                nc.vector.memset(slab[:, 0:1, :], 0.5)

            sl_u = slab[:, :, 0:FU]
            Ms = slabs.tile([P, g, FM], dt.bfloat16, tag="Ms")
            exv = Ms[:, :, FU:FM]
            if no_score:
                nc.vector.memset(exv, 1.0)
            else:
                tt = slabs.tile([P, g, FU], dt.bfloat16, tag="tt")
                nc.vector.tensor_tensor(
                    tt[:], sl_u,
                    xr_b[:].unsqueeze(1).to_broadcast([P, g, FU]),
                    op.add)
                # leaky-relu on the Scalar engine, in place (frees a DVE
                # pass and a tile). ACT's Lrelu hardwires slope 0.01
                # (ignores alpha); Prelu honors it
                nc.scalar.activation(tt[:], tt[:], AF.Prelu, alpha=0.2)
                # vv and the halving tree run in fp16: 16-bit keeps the DVE
                # 2x mode and the 10-bit mantissa keeps partial-sum rounding
                # out of the scores (values here are O(10), no range risk)
                vv = slabs.tile([P, g, FU], dt.float16, tag="vv")
                nc.vector.tensor_tensor(
                    vv[:], tt[:],
                    att_s[:].unsqueeze(1).to_broadcast([P, g, FU]),
                    op.mult)

                # per-head sum over C via a tensor_tensor halving tree (2x)
                # instead of a 1x tensor_reduce; levels below the first halve
                # in place within tr
                vh = vv[:].rearrange("p g (h c) -> p g h c", c=C)
                tr = slabs.tile([P, g, H, C // 2], dt.float16, tag="tr")
                nc.vector.tensor_tensor(
                    tr[:], vh[:, :, :, 0:C // 2], vh[:, :, :, C // 2:C],
                    op.add)
                lv = C // 2
                cur = tr[:]
                while lv > 2:
                    nc.vector.tensor_tensor(
                        cur[:, :, :, 0:lv // 2], cur[:, :, :, 0:lv // 2],
                        cur[:, :, :, lv // 2:lv], op.add)
                    cur = cur
                    lv //= 2
                cur = cur[:, :, :, 0:2]
                sc2 = small.tile([P, g, H], dt.float32, tag="sc2")
                scf = small.tile([P, g, H], dt.float32, tag="scf")
                nc.vector.tensor_tensor(
                    scf[:].unsqueeze(3), cur[:, :, :, 0:1], cur[:, :, :, 1:2],
                    op.add)
                nc.vector.tensor_tensor(
                    sc2[:], scf[:], mg[:].unsqueeze(2).to_broadcast([P, g, H]),
                    op.add)

                # scores are O(10) bounded -> exp without max subtraction
                nc.scalar.activation(exv, sc2[:], AF.Exp)
            nc.vector.tensor_tensor(
                Ms[:, :, 0:FU].rearrange("p g (h c) -> p g h c", c=C),
                sl_u.rearrange("p g (h c) -> p g h c", c=C),
                exv.unsqueeze(3).to_broadcast([P, g, H, C]),
                op.mult)

            # aggregate strata in groups of QG per matmul (wider rhs keeps
            # the PE streaming instead of paying per-instruction overhead)
            QG = 3 if layer == 1 else 7
            ngrp = min(QG, g)
            ps = psum.tile([P, ngrp, FM], dt.float32, tag="ps")
            nmm = (g + ngrp - 1) // ngrp
            for i in range(nmm):
                q0 = i * ngrp
                w = min(ngrp, g - q0)
                nc.tensor.matmul(ps[:, 0:w, :], id_s[:],
                                 Ms[:, q0:q0 + w, :],
                                 start=(i == 0), stop=(i == nmm - 1))
            # combine the ngrp partial groups (note: strata counts are even
            # multiples... tail group only wrote w<=ngrp slots on its last
            # matmul, but those slots were fully accumulated on earlier
            # passes, so every slot is valid)
            acc = small.tile([P, FM], dt.float32, tag="acc")
            nc.vector.tensor_copy(acc[:], ps[:, 0, :])
            for k in range(1, ngrp):
                nc.vector.tensor_tensor(acc[:], acc[:], ps[:, k, :], op.add)

            dn = small.tile([P, H], dt.float32, tag="dn")
            nc.vector.tensor_scalar_add(dn[:], acc[:, FU:FM], EPS)
            rd = small.tile([P, H], dt.float32, tag="rd")
            nc.vector.reciprocal(rd[:], dn[:])
            ov = small.tile([P, FU], dt.float32, tag="ov")
            nc.vector.tensor_tensor(
                ov[:].rearrange("p (h c) -> p h c", c=C),
                acc[:, 0:FU].rearrange("p (h c) -> p h c", c=C),
                rd[:].unsqueeze(2).to_broadcast([P, H, C]),
                op.mult)
            if layer == 2 and not no_tail:
                # bias-add writes straight into the persistent y buffer; all
                # log-softmax work happens batched after the loop
                nc.vector.tensor_tensor(y_all[:, j0, :], ov[:], bias_s[:],
                                        op.add)
            elif layer == 2:
                ob = small.tile([P, FU], out_dt, tag="ob")
                nc.vector.tensor_tensor(ob[:], ov[:], bias_s[:], op.add)
                nc.sync.dma_start(out_d[orow:orow + P, :], ob[:, 0:FOUT])
            else:
                # fused layer-2 transform: he = elu(h)+1, then
                # [xl2|xr2] = he @ W2cat + bias2' (bias pre-shifted by
                # -ones@W2 on the host)
                ob = small.tile([P, FU], dt.bfloat16, tag="ob")
                nc.vector.tensor_tensor(ob[:], ov[:], bias_s[:], op.add)
                mm2 = small.tile([P, FU], dt.bfloat16, tag="mm2")
                nc.vector.tensor_scalar_min(mm2[:], ob[:], 0.0)
                em2 = small.tile([P, FU], dt.bfloat16, tag="em2")
                nc.scalar.activation(em2[:], mm2[:], AF.Exp)
                he = small.tile([P, FU], dt.bfloat16, tag="he")
                nc.vector.scalar_tensor_tensor(he[:], ob[:], 0.0, em2[:],
                                               op.max, op.add)
                pst = psum.tile([P, P], dt.bfloat16, tag="pst")
                nc.tensor.transpose(pst[:], he[:], id_s[:])
                heT = small.tile([P, P], dt.bfloat16, tag="heT")
                nc.vector.tensor_copy(heT[:], pst[:])
                ps2 = psum.tile([P, P], dt.float32, tag="ps2")
                nc.tensor.matmul(ps2[:], heT[:], w2_s[:], start=True,
                                 stop=True)
                xl2_t = small.tile([P, F2P], dt.bfloat16, tag="xl2")
                nc.vector.tensor_tensor(xl2_t[:], ps2[:, 0:F2P],
                                        b2_s[:, 0:F2P], op.add)
                xr2_t = small.tile([P, F2P], dt.bfloat16, tag="xr2")
                nc.vector.tensor_tensor(xr2_t[:], ps2[:, F2P:P],
                                        b2_s[:, F2P:P], op.add)
                nc.sync.dma_start(xl2_d[orow:orow + P, 0:F2P], xl2_t[:])
                nc.sync.dma_start(xr2_d[orow:orow + P, :], xr2_t[:])

            oa += ga
            obi += gb
            om += g
            orow += P

        if layer == 2 and not no_tail:
            # batched log-softmax over all blocks: y_all holds [P, NB, 64]
            ya4 = y_all[:, 0:NB, :].rearrange("p n (h c) -> p n h c", c=8)
            yr = ya4[:, :, :, 0:DOUT]                  # [P, NB, H, 7]
            mx_all = persist.tile([P, NB], dt.float32)
            nc.vector.tensor_reduce(mx_all[:], yr, mybir.AxisListType.XY,
                                    op.max)
            ysub = persist.tile([P, NB, FU], dt.float32, name="ysub")
            nc.vector.tensor_tensor(
                ysub[:], y_all[:, 0:NB, :],
                mx_all[:].unsqueeze(2).to_broadcast([P, NB, FU]),
                op.subtract)
            et_all = persist.tile([P, NB, FU], dt.float32, name="et_all")
            nc.scalar.activation(et_all[:], ysub[:], AF.Exp)
            er4 = et_all[:].rearrange("p n (h c) -> p n h c", c=8)
            nc.vector.tensor_reduce(s_all[:], er4[:, :, :, 0:DOUT],
                                    mybir.AxisListType.XY, op.add)
            # ln(S) via exponent/mantissa split (no Ln in any HW act table):
            # ln(S) = (e - 127)*ln2 + poly(m), m in [1, 2)
            C5, C4, C3, C2, C1, C0 = (0.030102625011658456,
                                      -0.2806325404494927,
                                      1.1048082361987304,
                                      -2.4208125632180866,
                                      3.4982279012091095,
                                      -1.9316715417207186)
            bits = s_all[:].bitcast(dt.int32)
            ei = persist.tile([P, NB], dt.int32)
            nc.vector.tensor_scalar(ei[:], bits, 23, None,
                                    op.arith_shift_right)
            ef = persist.tile([P, NB], dt.float32)
            nc.vector.tensor_copy(ef[:], ei[:])
            mi = persist.tile([P, NB], dt.int32)
            nc.vector.tensor_scalar(mi[:], bits, 0x007FFFFF, 0x3F800000,
                                    op.bitwise_and, op.bitwise_or)
            mf = mi[:].bitcast(dt.float32)
            pp = persist.tile([P, NB], dt.float32)
            nc.vector.tensor_scalar(pp[:], mf, C5, C4, op.mult, op.add)
            qq = persist.tile([P, NB], dt.float32)
            for ck in (C3, C2, C1, C0):
                nc.vector.tensor_tensor(qq[:], pp[:], mf, op.mult)
                nc.vector.tensor_scalar_add(pp[:], qq[:], ck)
            # ct = mx + (e-127)*ln2 + poly(m)
            lnm = pp
            ct_all = persist.tile([P, NB], dt.float32)
            nc.vector.scalar_tensor_tensor(
                ct_all[:], ef[:], 0.6931471805599453, lnm[:],
                op.mult, op.add)
            lnS = persist.tile([P, NB], dt.float32)
            nc.vector.tensor_scalar_add(lnS[:], ct_all[:],
                                        -127.0 * 0.6931471805599453)
            # ysub already has -mx; subtract only ln(S)
            of_all = persist.tile([P, NB, FOUT], dt.float32, name="of_all")
            nc.vector.tensor_tensor(
                of_all[:].rearrange("p n (h c) -> p n h c", c=DOUT),
                ysub[:].rearrange("p n (h c) -> p n h c", c=8)[:, :, :, 0:DOUT],
                lnS[:].unsqueeze(2).unsqueeze(3).to_broadcast(
                    [P, NB, H, DOUT]),
                op.subtract)
            for j in range(NB):
                nc.sync.dma_start(out_d[j * P:(j + 1) * P, :],
                                  of_all[:, j, :])
    nc.compile()
    return nc


# ---------------------------------------------------------------------------
# runner
# ---------------------------------------------------------------------------

_state = {}


def _run(nc, in_maps, trace=False):
    from concourse.bass_utils import run_bass_kernel_spmd
    return run_bass_kernel_spmd(nc, in_maps, core_ids=list(range(NCORES)),
                                trace=trace)


def _bench_run(nc, in_maps):
    import bench
    results, wall, walls = bench.bench_neff(nc, in_maps)
    return results, wall, walls


def _bcast_rows(v, rows=P):
    """[n] -> [rows, n] replicated, contiguous."""
    return np.ascontiguousarray(np.broadcast_to(np.asarray(v)[None, :],
                                                (rows, len(v))))


def kernel(x, edge_index, Wl1, bl1, Wr1, br1, att1, bias1,
           Wl2, bl2, Wr2, br2, att2, bias2, _bench=False, _times=None,
           _walls=None):
    x = np.asarray(x, _f32)
    edge_index = np.asarray(edge_index)

    g = _prep_graph(edge_index)
    members, GA, GB = g["members"], g["GA"], g["GB"]

    ckey = (tuple(GA), tuple(GB))
    if _state.get("ckey") != ckey:
        _state["ckey"] = ckey
        _state["nc_t1"] = _build_transform(2 * F1, F1, F1, F1, elu_in=False)
        _state["nc_e1"] = _build_edge(1, GA, GB, g["sumGA"], g["sumGB"], g["sumG"])
        _state["nc_e2"] = _build_edge(2, GA, GB, g["sumGA"], g["sumGB"], g["sumG"])

    id128 = np.eye(P, dtype=_bf16)

    def gather_nodes(arr, mem):
        flat = mem.reshape(-1)
        out = arr[np.maximum(flat, 0)]
        out[flat < 0] = 0
        return out

    def trace_run(key, nc, in_maps):
        if _bench:
            results, wall, wl = _bench_run(nc, in_maps)
            if _times is not None:
                _times[key] = wall
            if _walls is not None:
                _walls[key] = wl
            return results
        r = _run(nc, in_maps, trace=False)
        return r.results

    # ---- T1 ----
    W1 = np.concatenate([Wl1, Wr1], axis=1).astype(_bf16)      # [128, 256]
    B1 = np.concatenate([bl1, br1]).astype(_f32)               # [256]
    B1t = _bcast_rows(B1)
    t1_maps = []
    for k in range(NCORES):
        xg = gather_nodes(x, members[k])                       # [6272, 128]
        t1_maps.append({"xT": np.ascontiguousarray(xg.T).astype(_bf16),
                        "Wcat": W1, "Bcat": B1t})
    r1 = trace_run("t1", _state["nc_t1"], t1_maps)

    # assemble layer-1 gather table
    tab1 = np.zeros((NPAD, P), _bf16)
    for k in range(NCORES):
        flat = members[k].reshape(-1)
        ok = flat >= 0
        tab1[flat[ok]] = r1[k]["xl"][ok]
    tab1A = np.ascontiguousarray(tab1[:SPLIT])
    tab1B = np.ascontiguousarray(tab1[SPLIT:])

    # ---- E1 (fused with the layer-2 node transform) ----
    Wl2p = np.zeros((P, F2P), _f32)
    Wl2p.reshape(P, H, 8)[:, :, :DOUT] = np.asarray(Wl2, _f32).reshape(P, H, DOUT)
    Wr2p = np.zeros((P, F2P), _f32)
    Wr2p.reshape(P, H, 8)[:, :, :DOUT] = np.asarray(Wr2, _f32).reshape(P, H, DOUT)
    W2 = np.concatenate([Wl2p, Wr2p], axis=1).astype(_bf16)  # [128,128]
    bl2p = np.zeros(F2P, _f32)
    bl2p.reshape(H, 8)[:, :DOUT] = np.asarray(bl2, _f32).reshape(H, DOUT)
    br2p = np.zeros(F2P, _f32)
    br2p.reshape(H, 8)[:, :DOUT] = np.asarray(br2, _f32).reshape(H, DOUT)
    # e1 consumes elu(h)+1, so shift the bias by -ones@W (col sums of the
    # bf16-rounded W as actually used on-device)
    B2 = np.concatenate([bl2p, br2p]) - W2.astype(_f32).sum(axis=0)
    B2t = _bcast_rows(B2)

    att1_t = _bcast_rows(att1.reshape(-1)).astype(_bf16)       # [128, 128]
    bias1_t = _bcast_rows(bias1).astype(_f32)
    e1_maps = []
    for k in range(NCORES):
        e1_maps.append({"tabA": tab1A, "tabB": tab1B,
                        "xr": r1[k]["xr"],
                        "idxA": g["idxA"][k], "idxB": g["idxB"][k],
                        "mneg": g["mneg"][k],
                        "attT": att1_t, "biasT": bias1_t, "idT": id128,
                        "W2cat": W2, "bias2T": B2t})
    re1 = trace_run("e1", _state["nc_e1"], e1_maps)

    tab2 = np.zeros((NPAD, P), _bf16)
    for k in range(NCORES):
        flat = members[k].reshape(-1)
        ok = flat >= 0
        tab2[flat[ok]] = re1[k]["xl2"][ok]
    tab2A = np.ascontiguousarray(tab2[:SPLIT])
    tab2B = np.ascontiguousarray(tab2[SPLIT:])

    # ---- E2 ----
    att2p = np.zeros((H, 8), _f32)
    att2p[:, :DOUT] = np.asarray(att2, _f32)
    att2_t = _bcast_rows(att2p.reshape(-1)).astype(_bf16)      # [128, 64]
    bias2p = np.zeros(F2P, _f32)
    bias2p.reshape(H, 8)[:, :DOUT] = np.asarray(bias2, _f32).reshape(H, DOUT)
    bias2_t = _bcast_rows(bias2p)
    e2_maps = []
    for k in range(NCORES):
        e2_maps.append({"tabA": tab2A, "tabB": tab2B,
                        "xr": re1[k]["xr2"],
                        "idxA": g["idxA"][k], "idxB": g["idxB"][k],
                        "mneg": g["mneg"][k],
                        "attT": att2_t, "biasT": bias2_t, "idT": id128})
    re2 = trace_run("e2", _state["nc_e2"], e2_maps)

    out = np.zeros((N, H * DOUT), _f32)
    for k in range(NCORES):
        flat = members[k].reshape(-1)
        ok = flat >= 0
        out[flat[ok]] = re2[k]["out"][ok]
    return out



# revision 59
# speedup vs baseline: 1.5353x; 1.2418x over previous
"""GATv2 (2-layer, 8-head) Trainium2 kernel, 8-core node-sharded.

Pipeline per layer:
  T-NEFF (per-core, sharded): node transforms xl = x@Wl+bl, xr = x@Wr+br
    via fp32r matmuls; emits bf16 gather tables (xl) and bf16 xr shards.
  host: assembles the full xl gather table from the 8 shards (data movement
    only), then
  E-NEFF (per-core, sharded by dst): per-edge score + segment-softmax +
    aggregate, with edges laid out stratum-major: edge slot (q, d) holds the
    q-th in-edge of dst-slot d, so partition index == dst slot.  The
    xr broadcast is a plain broadcast AP, segment aggregation is a PSUM
    accumulation of identity matmuls, and segment max/sum are free-dim
    reduces.  xl[src] rows are fetched with gpsimd dma_gather (int16
    indices, so the node table is split at 32768 and each block gathers
    from both halves into disjoint strata).

Between the two layers the host only concatenates/transposes shards.
"""

import os
from contextlib import ExitStack

import ml_dtypes
import numpy as np

N, E0, DIN, H, DH, DOUT = 50000, 1600000, 128, 8, 16, 7
F1 = H * DH            # 128
F2P = 64               # layer-2 per-node feature block, 8 heads x 8 (7 real)
NCORES = 8
P = 128
NBLK = 392             # 392*128 = 50176 >= N, 392 % 8 == 0
NB = NBLK // NCORES    # 49 blocks per core
NOWN = NB * P          # 6272 nodes per core (incl. pad slots)
NPAD = NBLK * P        # 50176
SPLIT = 32768
TABB_ROWS = NPAD - SPLIT  # 17408
NEG = -60.0  # mask for padded strata; scores are O(10) so exp(-60+s) == 0
EPS = 1e-16

_f32 = np.float32
_bf16 = ml_dtypes.bfloat16


# ---------------------------------------------------------------------------
# host-side graph preprocessing (pure index/layout manipulation)
# ---------------------------------------------------------------------------

def _prep_graph(edge_index):
    src = np.concatenate([edge_index[0], np.arange(N, dtype=np.int64)])
    dst = np.concatenate([edge_index[1], np.arange(N, dtype=np.int64)])
    src = src.astype(np.int64)
    dst = dst.astype(np.int64)

    low = src < SPLIT
    l_cnt = np.bincount(dst[low], minlength=N).astype(np.int64)
    h_cnt = np.bincount(dst[~low], minlength=N).astype(np.int64)

    # group nodes into blocks of 128 with near-equal (low-deg, high-deg):
    # primary sort by low-half in-degree, then re-sort h within coarse
    # l-bands so both per-window maxima stay tight (pads sumG 1994->1854)
    order = np.lexsort((h_cnt, l_cnt))
    BAND = 8192
    parts = []
    for s in range(0, N, BAND):
        seg = order[s:s + BAND]
        parts.append(seg[np.argsort(h_cnt[seg], kind="stable")])
    order = np.concatenate(parts)
    nodes_sorted = np.concatenate([order, np.full(NPAD - N, -1, np.int64)])
    blocks = nodes_sorted.reshape(NBLK, P)          # [392, 128]

    l_blk = np.where(blocks >= 0, l_cnt[np.maximum(blocks, 0)], 0).max(axis=1)
    h_blk = np.where(blocks >= 0, h_cnt[np.maximum(blocks, 0)], 0).max(axis=1)
    # block-slot j on every core runs global blocks j*8+k; shared strata counts
    GA = l_blk.reshape(NB, NCORES).max(axis=1).astype(int)   # [49]
    GB = h_blk.reshape(NB, NCORES).max(axis=1).astype(int)
    GA = GA.astype(int)
    GB = GB.astype(int)

    # per-node padded src lists, split by src half
    key = dst * 2 + (~low).astype(np.int64)
    oe = np.argsort(key, kind="stable")
    ss, sk = src[oe], key[oe]
    starts = np.searchsorted(sk, np.arange(2 * N))
    pos = np.arange(len(ss)) - starts[sk]
    Amax = max(int(l_cnt.max()), int(GA.max()))
    Bmax = max(int(h_cnt.max()), int(GB.max()))
    A_pad = np.zeros((N, Amax), np.int32)
    B_pad = np.zeros((N, Bmax), np.int32)
    am = (sk % 2) == 0
    A_pad[sk[am] // 2, pos[am]] = ss[am]
    B_pad[sk[~am] // 2, pos[~am]] = ss[~am] - SPLIT

    sumGA, sumGB = int(GA.sum()), int(GB.sum())
    sumG = sumGA + sumGB

    members = [None] * NCORES
    idxA = [None] * NCORES
    idxB = [None] * NCORES
    mneg = [None] * NCORES

    for k in range(NCORES):
        mem = blocks[np.arange(NB) * NCORES + k]       # [49, 128]
        members[k] = mem
        ia = np.zeros((P, 8 * sumGA), np.int16)
        ib = np.zeros((P, 8 * sumGB), np.int16)
        mg = np.full((P, sumG), NEG, _f32)
        oa = ob = om = 0
        for j in range(NB):
            ga, gb = GA[j], GB[j]
            m = mem[j]
            msafe = np.maximum(m, 0)
            larr = np.where(m >= 0, l_cnt[msafe], 0)
            harr = np.where(m >= 0, h_cnt[msafe], 0)
            if ga:
                plane = A_pad[msafe, :ga]              # [128, ga] (d, q)
                flat = plane.T.reshape(-1)             # slot-major (q, d)
                ia[:, 8 * oa:8 * (oa + ga)] = np.tile(
                    flat.reshape(-1, 16).T, (8, 1)).astype(np.int16)
                mg[:, om:om + ga] = np.where(
                    np.arange(ga)[None, :] < larr[:, None], 0.0, NEG)
            if gb:
                plane = B_pad[msafe, :gb]
                flat = plane.T.reshape(-1)
                ib[:, 8 * ob:8 * (ob + gb)] = np.tile(
                    flat.reshape(-1, 16).T, (8, 1)).astype(np.int16)
                mg[:, om + ga:om + ga + gb] = np.where(
                    np.arange(gb)[None, :] < harr[:, None], 0.0, NEG)
            oa += ga
            ob += gb
            om += ga + gb
        idxA[k], idxB[k], mneg[k] = ia, ib, mg

    return dict(members=members, GA=GA, GB=GB, idxA=idxA, idxB=idxB,
                mneg=mneg, sumGA=sumGA, sumGB=sumGB, sumG=sumG)


# ---------------------------------------------------------------------------
# NEFF builders
# ---------------------------------------------------------------------------

def _mk_bass(num_swdge_queues=1):
    import concourse.bacc as bacc
    return bacc.Bacc("TRN2", target_bir_lowering=False,
                     num_swdge_queues=num_swdge_queues)


def _build_transform(fo, xl_cols, xl_w, xr_w, elu_in, repeat=1):
    """xT [128, NOWN] (bf16) @ Wcat [128, fo] -> xl rows + xr rows (bf16).

    xl tensor is [NOWN, xl_cols]; only cols [0:xl_w] are written (rest
    stays zero).  xr tensor is [NOWN, xr_w].  With elu_in, the input is
    layer-1's raw (pre-activation) output h and the matmul consumes
    elu(h)+1 -- the host subtracts ones@W from the bias to compensate."""
    import concourse.mybir as mybir
    import concourse.tile as tile

    nc = _mk_bass()
    BF16, F32 = mybir.dt.bfloat16, mybir.dt.float32
    AF = mybir.ActivationFunctionType
    op = mybir.AluOpType
    xT = nc.dram_tensor("xT", [P, NOWN], BF16, kind="ExternalInput")
    W = nc.dram_tensor("Wcat", [P, fo], BF16, kind="ExternalInput")
    B = nc.dram_tensor("Bcat", [P, fo], F32, kind="ExternalInput")
    xl = nc.dram_tensor("xl", [NOWN, xl_cols], BF16, kind="ExternalOutput")
    xr = nc.dram_tensor("xr", [NOWN, xr_w], BF16, kind="ExternalOutput")

    with tile.TileContext(nc) as tc, ExitStack() as ctx:
        const = ctx.enter_context(tc.tile_pool(name="const", bufs=1))
        work = ctx.enter_context(tc.tile_pool(name="work", bufs=3))
        psum = ctx.enter_context(tc.tile_pool(name="psum", bufs=2, space="PSUM"))

        w_s = const.tile([P, fo], BF16)
        nc.sync.dma_start(w_s[:], W[:, :])
        b_s = const.tile([P, fo], F32)
        nc.sync.dma_start(b_s[:], B[:, :])
        # one big input DMA instead of 49 small ones (per-DMA fixed cost)
        xin = const.tile([P, NOWN], BF16)
        nc.sync.dma_start(xin[:], xT[:, :])

        olb = const.tile([P, NB, xl_w], BF16)
        orb = const.tile([P, NB, xr_w], BF16)
        for j0 in range(NB * repeat):
            j = j0 % NB
            lhs = xin[:, j * P:(j + 1) * P]
            if elu_in:
                # elu(h)+1 = max(h,0) + exp(min(h,0))
                mm = work.tile([P, P], BF16, tag="mm")
                nc.vector.tensor_scalar_min(mm[:], lhs, 0.0)
                em = work.tile([P, P], BF16, tag="em")
                nc.scalar.activation(em[:], mm[:], AF.Exp)
                lhs2 = work.tile([P, P], BF16, tag="lhs2")
                nc.vector.scalar_tensor_tensor(lhs2[:], lhs, 0.0, em[:],
                                               op.max, op.add)
                lhs = lhs2[:]
            ps = psum.tile([P, fo], F32, tag="ps")
            nc.tensor.matmul(ps[:], lhs, w_s[:], start=True, stop=True)
            nc.vector.tensor_tensor(olb[:, j, :], ps[:, 0:xl_w],
                                    b_s[:, 0:xl_w], op.add)
            nc.vector.tensor_tensor(orb[:, j, :], ps[:, xl_w:fo],
                                    b_s[:, xl_w:fo], op.add)
        # two batched output DMAs (row j*128+p <- tile[p, j, :])
        nc.sync.dma_start(
            xl[:, 0:xl_w].rearrange("(n p) w -> p n w", p=P), olb[:])
        nc.sync.dma_start(
            xr[:, :].rearrange("(n p) w -> p n w", p=P), orb[:])
    nc.compile()
    return nc


def _build_edge(layer, GA, GB, sumGA, sumGB, sumG,
                no_tail=False, repeat=1, no_gather=False, no_score=False):
    """Edge phase for one layer (see module docstring).  no_tail/repeat/
    no_gather/no_score are timing-diagnostic variants (wrong results)."""
    import concourse.bass as bass
    import concourse.mybir as mybir
    import concourse.tile as tile
    from concourse import library_config

    FU = F1 if layer == 1 else F2P      # used feature cols (128 / 64)
    C = DH if layer == 1 else 8         # per-head cols in slab (16 / 8)
    FM = FU + H                         # matmul rhs cols (agg | denom)
    FOUT = F1 if layer == 1 else H * DOUT

    # 4 SWDGE queues: dma_gather descriptor generation runs on the Q7 core
    # pair (2q, 2q+1) selected by queue_num -- round-robinning the gathers
    # across queues 0-3 parallelizes descgen over all 8 Q7 cores instead of
    # serializing on cores 0/1
    nc = _mk_bass(num_swdge_queues=4)
    dt = mybir.dt
    op = mybir.AluOpType
    AF = mybir.ActivationFunctionType

    tabA = nc.dram_tensor("tabA", [SPLIT, P], dt.bfloat16, kind="ExternalInput")
    tabB = nc.dram_tensor("tabB", [TABB_ROWS, P], dt.bfloat16, kind="ExternalInput")
    xr_d = nc.dram_tensor("xr", [NOWN, FU], dt.bfloat16, kind="ExternalInput")
    idxA = nc.dram_tensor("idxA", [P, 8 * sumGA], dt.int16, kind="ExternalInput")
    idxB = nc.dram_tensor("idxB", [P, 8 * sumGB], dt.int16, kind="ExternalInput")
    mneg = nc.dram_tensor("mneg", [P, sumG], dt.float32, kind="ExternalInput")
    attT = nc.dram_tensor("attT", [P, FU], dt.bfloat16, kind="ExternalInput")
    biasT = nc.dram_tensor("biasT", [P, FU], dt.float32, kind="ExternalInput")
    idT = nc.dram_tensor("idT", [P, P], dt.bfloat16, kind="ExternalInput")
    out_dt = dt.bfloat16 if layer == 1 else dt.float32
    if layer == 1:
        # layer 1 fuses the layer-2 node transform: instead of h it emits
        # xl2 gather-table rows and xr2 directly (t2's NEFF disappears)
        W2c = nc.dram_tensor("W2cat", [P, P], dt.bfloat16,
                             kind="ExternalInput")
        b2T = nc.dram_tensor("bias2T", [P, P], dt.float32,
                             kind="ExternalInput")
        xl2_d = nc.dram_tensor("xl2", [NOWN, P], dt.bfloat16,
                               kind="ExternalOutput")
        xr2_d = nc.dram_tensor("xr2", [NOWN, F2P], dt.bfloat16,
                               kind="ExternalOutput")
        out_d = None
    else:
        out_d = nc.dram_tensor("out", [NOWN, FOUT], out_dt,
                               kind="ExternalOutput")

    with tile.TileContext(nc) as tc, ExitStack() as ctx:
        const = ctx.enter_context(tc.tile_pool(name="const", bufs=1))
        io = ctx.enter_context(tc.tile_pool(name="io", bufs=4))
        slabp = ctx.enter_context(tc.tile_pool(name="slabp", bufs=3))
        slabs = ctx.enter_context(tc.tile_pool(name="slabs", bufs=2))
        psum = ctx.enter_context(tc.tile_pool(name="psum", bufs=2, space="PSUM"))
        small = ctx.enter_context(tc.tile_pool(name="small", bufs=2))

        nc.gpsimd.load_library(library_config.mlp)

        regcache = {}

        def nreg(v):
            if v not in regcache:
                regcache[v] = nc.gpsimd.to_reg(v)
            return regcache[v]

        att_s = const.tile([P, FU], dt.bfloat16)
        nc.sync.dma_start(att_s[:], attT[:, :])
        bias_s = const.tile([P, FU], dt.float32)
        nc.sync.dma_start(bias_s[:], biasT[:, :])
        id_s = const.tile([P, P], dt.bfloat16)
        nc.sync.dma_start(id_s[:], idT[:, :])
        if layer == 1:
            w2_s = const.tile([P, P], dt.bfloat16)
            nc.sync.dma_start(w2_s[:], W2c[:, :])
            b2_s = const.tile([P, P], dt.float32)
            nc.sync.dma_start(b2_s[:], b2T[:, :])

        if layer == 2:
            persist = ctx.enter_context(tc.tile_pool(name="persist", bufs=1))
            s_all = persist.tile([P, NB], dt.float32)
            y_all = persist.tile([P, NB * repeat, FU], dt.float32,
                                 name="y_all")
        else:
            persist = ctx.enter_context(tc.tile_pool(name="persist", bufs=1))
            h_all = persist.tile([P, NB * repeat, FU], dt.bfloat16,
                                 name="h_all")

        oa = obi = om = orow = 0
        for j0 in range(NB * repeat):
            j = j0 % NB
            if j == 0:
                oa = obi = om = orow = 0
            ga, gb = int(GA[j]), int(GB[j])
            g = ga + gb
            assert g > 0

            xr_b = io.tile([P, FU], dt.bfloat16, tag="xr")
            nc.sync.dma_start(xr_b[:], xr_d[j * P:(j + 1) * P, :])
            mg = io.tile([P, g], dt.float32, tag="mg")
            nc.sync.dma_start(mg[:], mneg[:, om:om + g])

            slab = slabp.tile([P, g, P], dt.bfloat16, tag="slab")
            if ga and not no_gather:
                ia = io.tile([P, 8 * ga], dt.int16, tag="ia")
                nc.sync.dma_start(ia[:], idxA[:, 8 * oa:8 * (oa + ga)])
                nc.gpsimd.dma_gather(slab[:, 0:ga, :], tabA[:, :], ia[:],
                                     P * ga, nreg(P * ga), P,
                                     single_packet=False,
                                     queue_num=(2 * j) % 4)
            if gb and not no_gather:
                ib = io.tile([P, 8 * gb], dt.int16, tag="ib")
                nc.sync.dma_start(ib[:], idxB[:, 8 * obi:8 * (obi + gb)])
                nc.gpsimd.dma_gather(slab[:, ga:g, :], tabB[:, :], ib[:],
                                     P * gb, nreg(P * gb), P,
                                     single_packet=False,
                                     queue_num=(2 * j + 1) % 4)
            if no_gather:
                nc.vector.memset(slab[:, 0:1, :], 0.5)

            sl_u = slab[:, :, 0:FU]
            Ms = slabs.tile([P, g, FM], dt.bfloat16, tag="Ms")
            exv = Ms[:, :, FU:FM]
            if no_score:
                nc.vector.memset(exv, 1.0)
            else:
                tt = slabs.tile([P, g, FU], dt.bfloat16, tag="tt")
                nc.vector.tensor_tensor(
                    tt[:], sl_u,
                    xr_b[:].unsqueeze(1).to_broadcast([P, g, FU]),
                    op.add)
                # leaky-relu on the Scalar engine, in place (frees a DVE
                # pass and a tile). ACT's Lrelu hardwires slope 0.01
                # (ignores alpha); Prelu honors it
                nc.scalar.activation(tt[:], tt[:], AF.Prelu, alpha=0.2)
                # vv and the halving tree run in fp16: 16-bit keeps the DVE
                # 2x mode and the 10-bit mantissa keeps partial-sum rounding
                # out of the scores (values here are O(10), no range risk)
                vv = slabs.tile([P, g, FU], dt.float16, tag="vv")
                nc.vector.tensor_tensor(
                    vv[:], tt[:],
                    att_s[:].unsqueeze(1).to_broadcast([P, g, FU]),
                    op.mult)

                # per-head sum over C via a tensor_tensor halving tree (2x)
                # instead of a 1x tensor_reduce; every level halves in place
                # within vv (out slice == first input slice, elementwise)
                vh = vv[:].rearrange("p g (h c) -> p g h c", c=C)
                lv = C
                while lv > 2:
                    nc.vector.tensor_tensor(
                        vh[:, :, :, 0:lv // 2], vh[:, :, :, 0:lv // 2],
                        vh[:, :, :, lv // 2:lv], op.add)
                    lv //= 2
                cur = vh[:, :, :, 0:2]
                sc2 = small.tile([P, g, H], dt.float32, tag="sc2")
                scf = small.tile([P, g, H], dt.float32, tag="scf")
                nc.vector.tensor_tensor(
                    scf[:].unsqueeze(3), cur[:, :, :, 0:1], cur[:, :, :, 1:2],
                    op.add)
                nc.vector.tensor_tensor(
                    sc2[:], scf[:], mg[:].unsqueeze(2).to_broadcast([P, g, H]),
                    op.add)

                # scores are O(10) bounded -> exp without max subtraction
                nc.scalar.activation(exv, sc2[:], AF.Exp)
            nc.vector.tensor_tensor(
                Ms[:, :, 0:FU].rearrange("p g (h c) -> p g h c", c=C),
                sl_u.rearrange("p g (h c) -> p g h c", c=C),
                exv.unsqueeze(3).to_broadcast([P, g, H, C]),
                op.mult)

            # aggregate strata in groups of QG per matmul (wider rhs keeps
            # the PE streaming instead of paying per-instruction overhead)
            QG = 3 if layer == 1 else 7
            ngrp = min(QG, g)
            ps = psum.tile([P, ngrp, FM], dt.float32, tag="ps")
            nmm = (g + ngrp - 1) // ngrp
            for i in range(nmm):
                q0 = i * ngrp
                w = min(ngrp, g - q0)
                nc.tensor.matmul(ps[:, 0:w, :], id_s[:],
                                 Ms[:, q0:q0 + w, :],
                                 start=(i == 0), stop=(i == nmm - 1))
            # combine the ngrp partial groups (note: strata counts are even
            # multiples... tail group only wrote w<=ngrp slots on its last
            # matmul, but those slots were fully accumulated on earlier
            # passes, so every slot is valid)
            acc = small.tile([P, FM], dt.float32, tag="acc")
            nc.vector.tensor_copy(acc[:], ps[:, 0, :])
            for k in range(1, ngrp):
                nc.vector.tensor_tensor(acc[:], acc[:], ps[:, k, :], op.add)

            dn = small.tile([P, H], dt.float32, tag="dn")
            nc.vector.tensor_scalar_add(dn[:], acc[:, FU:FM], EPS)
            rd = small.tile([P, H], dt.float32, tag="rd")
            nc.vector.reciprocal(rd[:], dn[:])
            ov = small.tile([P, FU], dt.float32, tag="ov")
            nc.vector.tensor_tensor(
                ov[:].rearrange("p (h c) -> p h c", c=C),
                acc[:, 0:FU].rearrange("p (h c) -> p h c", c=C),
                rd[:].unsqueeze(2).to_broadcast([P, H, C]),
                op.mult)
            if layer == 2 and not no_tail:
                # bias-add writes straight into the persistent y buffer; all
                # log-softmax work happens batched after the loop
                nc.vector.tensor_tensor(y_all[:, j0, :], ov[:], bias_s[:],
                                        op.add)
            elif layer == 2:
                ob = small.tile([P, FU], out_dt, tag="ob")
                nc.vector.tensor_tensor(ob[:], ov[:], bias_s[:], op.add)
                nc.sync.dma_start(out_d[orow:orow + P, :], ob[:, 0:FOUT])
            else:
                # stash raw h; the fused layer-2 transform runs as a batched
                # suffix after the loop (a mid-chain PE transpose here would
                # stall the in-order PE stream behind slow DVE deps)
                nc.vector.tensor_tensor(h_all[:, j0, :], ov[:], bias_s[:],
                                        op.add)

            oa += ga
            obi += gb
            om += g
            orow += P

        if layer == 1:
            # batched fused layer-2 node transform:
            # he = elu(h)+1 (batched over all blocks), then per block
            # transpose -> [xl2|xr2] = heT.T @ W2cat + bias2'
            nhb = NB * repeat
            CH = 7
            xl2_b = persist.tile([P, NB, F2P], dt.bfloat16, name="xl2b")
            xr2_b = persist.tile([P, NB, F2P], dt.bfloat16, name="xr2b")
            for c0 in range(0, nhb, CH):
                cw = min(CH, nhb - c0)
                hs = h_all[:, c0:c0 + cw, :]
                he_c = small.tile([P, cw, FU], dt.bfloat16, tag="heC")
                nc.vector.tensor_scalar_min(he_c[:], hs, 0.0)
                nc.scalar.activation(he_c[:], he_c[:], AF.Exp)
                nc.vector.scalar_tensor_tensor(he_c[:], hs, 0.0, he_c[:],
                                               op.max, op.add)
                for jj in range(cw):
                    j0 = c0 + jj
                    j = j0 % NB
                    pst = psum.tile([P, P], dt.bfloat16, tag="pst")
                    nc.tensor.transpose(pst[:], he_c[:, jj, :], id_s[:])
                    heT = small.tile([P, P], dt.bfloat16, tag="heT")
                    nc.vector.tensor_copy(heT[:], pst[:])
                    ps2 = psum.tile([P, P], dt.float32, tag="ps2")
                    nc.tensor.matmul(ps2[:], heT[:], w2_s[:], start=True,
                                     stop=True)
                    nc.vector.tensor_tensor(xl2_b[:, j, :], ps2[:, 0:F2P],
                                            b2_s[:, 0:F2P], op.add)
                    nc.vector.tensor_tensor(xr2_b[:, j, :], ps2[:, F2P:P],
                                            b2_s[:, F2P:P], op.add)
            nc.sync.dma_start(
                xl2_d[:, 0:F2P].rearrange("(n p) w -> p n w", p=P), xl2_b[:])
            nc.sync.dma_start(
                xr2_d[:, :].rearrange("(n p) w -> p n w", p=P), xr2_b[:])

        if layer == 2 and not no_tail:
            # batched log-softmax over all blocks: y_all holds [P, NB, 64]
            ya4 = y_all[:, 0:NB, :].rearrange("p n (h c) -> p n h c", c=8)
            yr = ya4[:, :, :, 0:DOUT]                  # [P, NB, H, 7]
            mx_all = persist.tile([P, NB], dt.float32)
            nc.vector.tensor_reduce(mx_all[:], yr, mybir.AxisListType.XY,
                                    op.max)
            ysub = persist.tile([P, NB, FU], dt.float32, name="ysub")
            nc.vector.tensor_tensor(
                ysub[:], y_all[:, 0:NB, :],
                mx_all[:].unsqueeze(2).to_broadcast([P, NB, FU]),
                op.subtract)
            et_all = persist.tile([P, NB, FU], dt.float32, name="et_all")
            nc.scalar.activation(et_all[:], ysub[:], AF.Exp)
            er4 = et_all[:].rearrange("p n (h c) -> p n h c", c=8)
            nc.vector.tensor_reduce(s_all[:], er4[:, :, :, 0:DOUT],
                                    mybir.AxisListType.XY, op.add)
            # ln(S) via exponent/mantissa split (no Ln in any HW act table):
            # ln(S) = (e - 127)*ln2 + poly(m), m in [1, 2)
            C5, C4, C3, C2, C1, C0 = (0.030102625011658456,
                                      -0.2806325404494927,
                                      1.1048082361987304,
                                      -2.4208125632180866,
                                      3.4982279012091095,
                                      -1.9316715417207186)
            bits = s_all[:].bitcast(dt.int32)
            ei = persist.tile([P, NB], dt.int32)
            nc.vector.tensor_scalar(ei[:], bits, 23, None,
                                    op.arith_shift_right)
            ef = persist.tile([P, NB], dt.float32)
            nc.vector.tensor_copy(ef[:], ei[:])
            mi = persist.tile([P, NB], dt.int32)
            nc.vector.tensor_scalar(mi[:], bits, 0x007FFFFF, 0x3F800000,
                                    op.bitwise_and, op.bitwise_or)
            mf = mi[:].bitcast(dt.float32)
            pp = persist.tile([P, NB], dt.float32)
            nc.vector.tensor_scalar(pp[:], mf, C5, C4, op.mult, op.add)
            qq = persist.tile([P, NB], dt.float32)
            for ck in (C3, C2, C1, C0):
                nc.vector.tensor_tensor(qq[:], pp[:], mf, op.mult)
                nc.vector.tensor_scalar_add(pp[:], qq[:], ck)
            # ct = mx + (e-127)*ln2 + poly(m)
            lnm = pp
            ct_all = persist.tile([P, NB], dt.float32)
            nc.vector.scalar_tensor_tensor(
                ct_all[:], ef[:], 0.6931471805599453, lnm[:],
                op.mult, op.add)
            lnS = persist.tile([P, NB], dt.float32)
            nc.vector.tensor_scalar_add(lnS[:], ct_all[:],
                                        -127.0 * 0.6931471805599453)
            # ysub already has -mx; subtract only ln(S)
            of_all = persist.tile([P, NB, FOUT], dt.float32, name="of_all")
            nc.vector.tensor_tensor(
                of_all[:].rearrange("p n (h c) -> p n h c", c=DOUT),
                ysub[:].rearrange("p n (h c) -> p n h c", c=8)[:, :, :, 0:DOUT],
                lnS[:].unsqueeze(2).unsqueeze(3).to_broadcast(
                    [P, NB, H, DOUT]),
                op.subtract)
            nc.sync.dma_start(
                out_d[:, :].rearrange("(n p) w -> p n w", p=P), of_all[:])
    nc.compile()
    return nc


# ---------------------------------------------------------------------------
# runner
# ---------------------------------------------------------------------------

_state = {}


def _run(nc, in_maps, trace=False):
    from concourse.bass_utils import run_bass_kernel_spmd
    return run_bass_kernel_spmd(nc, in_maps, core_ids=list(range(NCORES)),
                                trace=trace)


def _bench_run(nc, in_maps):
    import bench
    results, wall, walls = bench.bench_neff(nc, in_maps)
    return results, wall, walls


def _bcast_rows(v, rows=P):
    """[n] -> [rows, n] replicated, contiguous."""
    return np.ascontiguousarray(np.broadcast_to(np.asarray(v)[None, :],
                                                (rows, len(v))))


def kernel(x, edge_index, Wl1, bl1, Wr1, br1, att1, bias1,
           Wl2, bl2, Wr2, br2, att2, bias2, _bench=False, _times=None,
           _walls=None):
    x = np.asarray(x, _f32)
    edge_index = np.asarray(edge_index)

    g = _prep_graph(edge_index)
    members, GA, GB = g["members"], g["GA"], g["GB"]

    ckey = (tuple(GA), tuple(GB))
    if _state.get("ckey") != ckey:
        _state["ckey"] = ckey
        _state["nc_t1"] = _build_transform(2 * F1, F1, F1, F1, elu_in=False)
        _state["nc_e1"] = _build_edge(1, GA, GB, g["sumGA"], g["sumGB"], g["sumG"])
        _state["nc_e2"] = _build_edge(2, GA, GB, g["sumGA"], g["sumGB"], g["sumG"])

    id128 = np.eye(P, dtype=_bf16)

    def gather_nodes(arr, mem):
        flat = mem.reshape(-1)
        out = arr[np.maximum(flat, 0)]
        out[flat < 0] = 0
        return out

    def trace_run(key, nc, in_maps):
        if _bench:
            results, wall, wl = _bench_run(nc, in_maps)
            if _times is not None:
                _times[key] = wall
            if _walls is not None:
                _walls[key] = wl
            return results
        r = _run(nc, in_maps, trace=False)
        return r.results

    # ---- T1 ----
    W1 = np.concatenate([Wl1, Wr1], axis=1).astype(_bf16)      # [128, 256]
    B1 = np.concatenate([bl1, br1]).astype(_f32)               # [256]
    B1t = _bcast_rows(B1)
    t1_maps = []
    for k in range(NCORES):
        xg = gather_nodes(x, members[k])                       # [6272, 128]
        t1_maps.append({"xT": np.ascontiguousarray(xg.T).astype(_bf16),
                        "Wcat": W1, "Bcat": B1t})
    r1 = trace_run("t1", _state["nc_t1"], t1_maps)

    # assemble layer-1 gather table
    tab1 = np.zeros((NPAD, P), _bf16)
    for k in range(NCORES):
        flat = members[k].reshape(-1)
        ok = flat >= 0
        tab1[flat[ok]] = r1[k]["xl"][ok]
    tab1A = np.ascontiguousarray(tab1[:SPLIT])
    tab1B = np.ascontiguousarray(tab1[SPLIT:])

    # ---- E1 (fused with the layer-2 node transform) ----
    Wl2p = np.zeros((P, F2P), _f32)
    Wl2p.reshape(P, H, 8)[:, :, :DOUT] = np.asarray(Wl2, _f32).reshape(P, H, DOUT)
    Wr2p = np.zeros((P, F2P), _f32)
    Wr2p.reshape(P, H, 8)[:, :, :DOUT] = np.asarray(Wr2, _f32).reshape(P, H, DOUT)
    W2 = np.concatenate([Wl2p, Wr2p], axis=1).astype(_bf16)  # [128,128]
    bl2p = np.zeros(F2P, _f32)
    bl2p.reshape(H, 8)[:, :DOUT] = np.asarray(bl2, _f32).reshape(H, DOUT)
    br2p = np.zeros(F2P, _f32)
    br2p.reshape(H, 8)[:, :DOUT] = np.asarray(br2, _f32).reshape(H, DOUT)
    # e1 consumes elu(h)+1, so shift the bias by -ones@W (col sums of the
    # bf16-rounded W as actually used on-device)
    B2 = np.concatenate([bl2p, br2p]) - W2.astype(_f32).sum(axis=0)
    B2t = _bcast_rows(B2)

    att1_t = _bcast_rows(att1.reshape(-1)).astype(_bf16)       # [128, 128]
    bias1_t = _bcast_rows(bias1).astype(_f32)
    e1_maps = []
    for k in range(NCORES):
        e1_maps.append({"tabA": tab1A, "tabB": tab1B,
                        "xr": r1[k]["xr"],
                        "idxA": g["idxA"][k], "idxB": g["idxB"][k],
                        "mneg": g["mneg"][k],
                        "attT": att1_t, "biasT": bias1_t, "idT": id128,
                        "W2cat": W2, "bias2T": B2t})
    re1 = trace_run("e1", _state["nc_e1"], e1_maps)

    tab2 = np.zeros((NPAD, P), _bf16)
    for k in range(NCORES):
        flat = members[k].reshape(-1)
        ok = flat >= 0
        tab2[flat[ok]] = re1[k]["xl2"][ok]
    tab2A = np.ascontiguousarray(tab2[:SPLIT])
    tab2B = np.ascontiguousarray(tab2[SPLIT:])

    # ---- E2 ----
    att2p = np.zeros((H, 8), _f32)
    att2p[:, :DOUT] = np.asarray(att2, _f32)
    att2_t = _bcast_rows(att2p.reshape(-1)).astype(_bf16)      # [128, 64]
    bias2p = np.zeros(F2P, _f32)
    bias2p.reshape(H, 8)[:, :DOUT] = np.asarray(bias2, _f32).reshape(H, DOUT)
    bias2_t = _bcast_rows(bias2p)
    e2_maps = []
    for k in range(NCORES):
        e2_maps.append({"tabA": tab2A, "tabB": tab2B,
                        "xr": re1[k]["xr2"],
                        "idxA": g["idxA"][k], "idxB": g["idxB"][k],
                        "mneg": g["mneg"][k],
                        "attT": att2_t, "biasT": bias2_t, "idT": id128})
    re2 = trace_run("e2", _state["nc_e2"], e2_maps)

    out = np.zeros((N, H * DOUT), _f32)
    for k in range(NCORES):
        flat = members[k].reshape(-1)
        ok = flat >= 0
        out[flat[ok]] = re2[k]["out"][ok]
    return out

